# revision 1
# baseline (speedup 1.0000x reference)
"""TRN2 Bass kernel for nn_Attention_90460601189287.

Causal multi-head attention (B=2, N=2048, D=1024, H=16) with spectral-norm
(power-iteration) scaled qkv/proj dense layers, on 8 NeuronCores.

Sharding: tensor-parallel over heads. Core c owns heads {2c, 2c+1}: it gets
the matching 128 columns of each of W_qkv's q/k/v blocks and the matching
128 rows of W_proj, computes attention for its heads over the full batch,
and produces a partial y = x_att @ W_proj_rows. The host sums the 8
partials (the gather step for row-sharded matmul).

The tiny spectral-norm power-iteration scales (two matvecs + norms over the
weights, ~0.01% of total FLOPs; identical math to the reference:
sigma = ||W^T normalize(W u)||) are computed on host in fp32 during input
sharding; the resulting scalar scales are applied on-device.

Per-core device program (SPMD — identical program, per-core weight slices),
software-pipelined per 512-token window so PE/ACT/DVE overlap across stages:
  A: PE-transpose x into x^T; qkv^T = W^T x accumulated over 8 dm-chunks
     (float32r matmuls: tf32 rate, ~2e-4 final accuracy).
  A2: PE-transpose v^T into V-natural [k,128] tiles whose cols 64:128 are
      ones so the PV matmul also emits the softmax denominator.
  B: S^T = K Q^T per 128-k-block; exp(S - 30) on ScalarE (constant shift
     replaces the row-max pass; exact after normalization since scores are
     O(1)); causal mask multiply on diagonal blocks; O^T accumulated in
     PSUM with denominator on partitions 64:127, partition-aligned with
     the numerator; normalize via reciprocal+mult.
  C: y_partial = x_att^T-blocks @ W_proj, stored per half-window.
"""
from contextlib import ExitStack

import numpy as np

import concourse.bass as bass
import concourse.mybir as mybir
from concourse.bass_utils import run_bass_kernel_spmd
from concourse.masks import make_identity
from concourse.tile import TileContext

F32 = mybir.dt.float32
F32R = mybir.dt.float32r

N_CORES = 8
BATCH = 2
NTOK = 4096      # flattened b*n
D = 1024
NH = 2           # heads per core
HD = 64
B = 2
NSEQ = 2048
WQ = 512         # token window
NW = NTOK // WQ
NWB = NSEQ // WQ
KB = 128
SHIFT = 30.0


def r(ap):
    return ap.bitcast(F32R)


# ---------------------------------------------------------------------------
# Workaround: this walrus build accepts at most ONE sync wait per
# instruction. Hoist extra waits onto single-wait NOPs inserted before.
# ---------------------------------------------------------------------------
def _split_sync_waits(nc, max_waits=1):
    for f in nc.m.functions:
        for blk in f.blocks:
            insts = blk.instructions
            out = []
            changed = False
            for inst in insts:
                si = inst.sync_info
                waits = list(si.on_wait) if si is not None else []
                if len(waits) > max_waits:
                    extra = waits[:-max_waits]
                    for i in range(0, len(extra), max_waits):
                        nop = mybir.InstNoOp(name=f"I-{nc.next_id()}", ins=[],
                                             outs=[], engine=inst.engine)
                        nop.sync_info = mybir.SyncInfo(
                            on_wait=extra[i:i + max_waits], on_update=[])
                        nc.register_instruction(nop, overwrite=True)
                        out.append(nop)
                    si.on_wait = waits[-max_waits:]
                    inst.sync_info = si
                    changed = True
                out.append(inst)
            if changed:
                blk.instructions = out


class _TileContextSplit(TileContext):
    def __exit__(self, exc_type, exc_value, traceback):
        ret = super().__exit__(exc_type, exc_value, traceback)
        if exc_type is None:
            _split_sync_waits(self.nc)
        return ret


def declare_params(nc):
    x = nc.declare_dram_parameter("x", [NTOK, D], F32R, isOutput=False)
    wq = nc.declare_dram_parameter("wq", [D, NH * HD], F32, isOutput=False)
    wk = nc.declare_dram_parameter("wk", [D, NH * HD], F32, isOutput=False)
    wv = nc.declare_dram_parameter("wv", [D, NH * HD], F32, isOutput=False)
    wp = nc.declare_dram_parameter("wp", [NH * HD, D], F32, isOutput=False)
    cqk = nc.declare_dram_parameter("cqk", [128, 1], F32, isOutput=False)
    cv = nc.declare_dram_parameter("cv", [128, 1], F32, isOutput=False)
    cp = nc.declare_dram_parameter("cp", [128, 1], F32, isOutput=False)
    mask = nc.declare_dram_parameter("mask", [128, 896], F32, isOutput=False)
    y = nc.declare_dram_parameter("y", [NTOK, D], F32, isOutput=True)
    return x, wq, wk, wv, wp, cqk, cv, cp, mask, y


def _build_body(nc, tc):
    mm = r
    tr = lambda ap: ap
    psum_bufs = dict(tp=2, qkv=1, s=2, o=1)
    xt_split = 5
    x, wq, wk, wv, wp, cqk, cv, cp, mask, y = declare_params(nc)

    ctx = ExitStack()
    with ctx:
        singles = ctx.enter_context(tc.tile_pool(name="singles", bufs=1))
        ident = singles.tile([128, 128], F32)
        make_identity(nc, ident)
        # f32r-rounded identity: lets the vT transposes (whose data inputs
        # are already f32r-rounded) run at 1.5 cyc/row instead of 2.0
        ident_r = singles.tile([128, 128], F32)
        nc.vector.tensor_copy(r(ident_r[:]), ident[:])

        xw_pool = ctx.enter_context(tc.tile_pool(name="xw", bufs=2))
        xw_pre = {}

        def load_xw(w):
            if w == 0:
                subs = []
                for t in range(4):
                    xw_s = xw_pool.tile([128, D], F32R, tag="xw0",
                                        name="xw_s", bufs=4)
                    nc.sync.dma_start(
                        out=xw_s[:],
                        in_=x[w * WQ + t * 128:w * WQ + (t + 1) * 128, :])
                    subs.append(xw_s)
                xw_pre[w] = subs
            else:
                xw_t = xw_pool.tile([128, 4, D], F32R, tag="xw", name="xw_t")
                nc.sync.dma_start(
                    out=xw_t[:],
                    in_=x[w * WQ:(w + 1) * WQ, :]
                        .rearrange("(t p) d -> p t d", p=128))
                xw_pre[w] = xw_t

        load_xw(0)
        mask_sb = singles.tile([128, 896], F32)
        nc.gpsimd.dma_start(out=mask_sb[:], in_=mask[:])
        cqk_sb = singles.tile([128, 1], F32)
        nc.gpsimd.dma_start(out=cqk_sb[:], in_=cqk[:])
        cv_sb = singles.tile([128, 1], F32)
        nc.gpsimd.dma_start(out=cv_sb[:], in_=cv[:])
        cp_sb = singles.tile([128, 1], F32)
        nc.gpsimd.dma_start(out=cp_sb[:], in_=cp[:])
        shift_sb = singles.tile([128, 1], F32)
        nc.gpsimd.memset(shift_sb[:], -SHIFT)
        ones_sb = singles.tile([128, 4 * HD], F32)
        nc.gpsimd.memset(ones_sb[:], 1.0)
        zeros_sb = singles.tile([128, 384], F32)
        nc.gpsimd.memset(zeros_sb[:], 0.0)

        # weights first (small; the first qkv matmuls need chunk 0), then
        # the first x windows. Rounding copies are chunked so qkv chunk dm
        # is ready as soon as its three 64KB slices have landed.
        wq_sb = singles.tile([128, D], F32)
        wk_sb = singles.tile([128, D], F32)
        wv_sb = singles.tile([128, D], F32)
        wp_sb = singles.tile([128, D], F32)
        with tc.tile_pool(name="wst", bufs=1) as wst:
            wq_st = wst.tile([128, D], F32)
            wk_st = wst.tile([128, D], F32)
            wv_st = wst.tile([128, D], F32)
            wp_st = wst.tile([128, D], F32)
            for w_dram, w_st, w_fin in ((wq, wq_st, wq_sb),
                                         (wk, wk_st, wk_sb),
                                         (wv, wv_st, wv_sb)):
                nc.sync.dma_start(
                    out=w_st.rearrange("p (c m) -> p c m", c=8),
                    in_=w_dram.rearrange("(c p) m -> p c m", p=128))
                nc.vector.tensor_copy(mm(w_fin[:]), w_st[:])
            nc.sync.dma_start(out=wp_st[:], in_=wp[:])
            nc.vector.tensor_scalar_mul(mm(wp_sb[:]), wp_st[:], cp_sb[:, 0:1])

            load_xw(1)

        # per-window qkv^T and attention-output^T tiles (window granularity
        # is what lets stage B start while stage A is still running)
        qTw = [singles.tile([128, WQ], F32, name=f"qT_{w}") for w in range(NW)]
        kTw = [singles.tile([128, WQ], F32, name=f"kT_{w}") for w in range(NW)]
        vTw = [singles.tile([128, WQ], F32, name=f"vT_{w}") for w in range(NW)]
        xaw = [singles.tile([128, WQ], F32, name=f"xa_{w}") for w in range(NW)]
        # V natural layout per (head, batch, group of 4 k-blocks):
        # [128, 4, 128]; cols 64:128 all-ones (denominator trick)
        vnat = [[[singles.tile([128, 4, 2 * HD], F32, name=f"vn_{h}_{b}_{g}")
                  for g in range(NWB)] for b in range(B)] for h in range(NH)]

        # one PSUM pool for the whole kernel: tp1+qkv3+s2+o2 = 8 banks
        ps = ctx.enter_context(tc.tile_pool(name="ps", bufs=1, space="PSUM"))
        xt_pool = ctx.enter_context(tc.tile_pool(name="xt", bufs=3))
        a_pool = ctx.enter_context(tc.tile_pool(name="apool", bufs=6))
        den_pool = ctx.enter_context(tc.tile_pool(name="denpool", bufs=3))
        y_pool = ctx.enter_context(tc.tile_pool(name="ypool", bufs=2))

        # ---- Stage A for one token window, as a list of chunk closures so
        # the emitter can interleave them into stage B's PE bubbles ----
        def stage_a_ops(w):
            state = {}

            def start():
                if w not in xw_pre:
                    load_xw(w)
                state["xw"] = xw_pre.pop(w)
                state["qkv"] = [
                    ps.tile([128, WQ], F32, tag=f"qkv{i}", name=f"qkv_ps{i}",
                            bufs=psum_bufs["qkv"]) for i in range(3)]

            def mk_dm(dm):
                def op():
                    xw_t = state["xw"]
                    sub = isinstance(xw_t, list)
                    tp_ps = ps.tile([128, WQ], F32, tag="tp", name="tp_ps",
                                    bufs=psum_bufs["tp"])
                    for t in range(4):
                        xsrc = (xw_t[t][:, dm * 128:(dm + 1) * 128] if sub
                                else xw_t[:, t, dm * 128:(dm + 1) * 128])
                        nc.tensor.transpose(
                            r(tp_ps[:, t * 128:(t + 1) * 128]),
                            xsrc, r(ident_r[:]))
                    xt_t = xt_pool.tile([128, WQ], F32, tag="xt", name="xt_t")
                    if xt_split and dm % xt_split == 0:
                        nc.scalar.copy(mm(xt_t[:]), tp_ps[:])
                    else:
                        nc.vector.tensor_copy(mm(xt_t[:]), tp_ps[:])
                    for i, w_sb in enumerate((wq_sb, wk_sb, wv_sb)):
                        nc.tensor.matmul(state["qkv"][i][:],
                                         mm(w_sb[:, dm * 128:(dm + 1) * 128]),
                                         mm(xt_t[:]),
                                         start=(dm == 0), stop=(dm == 7))
                return op

            def copies():
                qkv_ps = state["qkv"]
                nc.vector.tensor_scalar_mul(mm(qTw[w][:]), qkv_ps[0][:],
                                            cqk_sb[:, 0:1])
                nc.scalar.copy(mm(kTw[w][:]), qkv_ps[1][:])
                nc.vector.tensor_scalar_mul(mm(vTw[w][:]), qkv_ps[2][:],
                                            cv_sb[:, 0:1])

            def mk_a2(h):
                def op():
                    b, g = divmod(w, NWB)
                    vn = vnat[h][b][g]
                    nc.vector.tensor_copy(
                        mm(vn[:, :, HD:2 * HD]),
                        ones_sb.rearrange("p (g d) -> p g d",
                                          g=4)[:, 0:4, 0:HD])
                    vp = ps.tile([128, 4, HD], F32, tag="tp", name="vp",
                                 bufs=psum_bufs["tp"])
                    for j in range(4):
                        nc.tensor.transpose(
                            r(vp[:, j, :]),
                            r(vTw[w][h * HD:(h + 1) * HD,
                                     j * KB:(j + 1) * KB]),
                            r(ident_r[h * HD:(h + 1) * HD,
                                      h * HD:(h + 1) * HD]))
                    nc.vector.tensor_copy(mm(vn[:, :, 0:HD]), vp[:])
                return op

            return ([start] + [mk_dm(dm) for dm in range(8)] + [copies] +
                    [mk_a2(h) for h in range(NH)])

        def stage_a(w):
            for op in stage_a_ops(w):
                op()

        # ---- Stage B for one (batch, q-window): both heads, interleaved by
        # k-block so two independent S->exp->PV chains hide the exp latency ----
        def stage_b(b, g, c_ops=()):
            c_iter = iter(c_ops)
            n_units = NH * (g + 1) * (WQ // KB)
            n_c = len(c_ops)
            emitted_c = 0
            done_units = 0
            for h in range(NH):
                hs = slice(h * HD, (h + 1) * HD)
                o_ps = ps.tile([2 * HD, WQ], F32, tag="o", name="o_ps",
                               bufs=psum_bufs["o"])
                nkb = (g + 1) * (WQ // KB)
                for kb in range(nkb):
                    kw = b * NWB + kb // 4       # global window of k block
                    ko = (kb % 4) * KB
                    sq = max(0, kb * KB - g * WQ)  # first causally-valid col
                    s_ps = ps.tile([128, WQ], F32, tag="s", name="s_ps",
                                   bufs=psum_bufs["s"])
                    nc.tensor.matmul(s_ps[:, sq:WQ],
                                     mm(kTw[kw][hs, ko:ko + KB]),
                                     mm(qTw[b * NWB + g][hs, sq:WQ]),
                                     start=True, stop=True)
                    a_t = a_pool.tile([128, WQ], F32, tag="a", name="a_t")
                    s = kb * KB - g * WQ
                    if s >= 0:  # diagonal block: causal masking. Columns
                        # [0:s] are entirely above the diagonal: zero them
                        # and restrict exp+mask to the valid range [s:512].
                        if s > 0:
                            nc.vector.tensor_copy(mm(a_t[:, 0:s]),
                                                  zeros_sb[:, 0:s])
                        nc.scalar.activation(mm(a_t[:, s:WQ]),
                                             s_ps[:, s:WQ],
                                             mybir.ActivationFunctionType.Exp,
                                             bias=shift_sb[:, 0:1], scale=1.0)
                        nc.vector.tensor_tensor(
                            out=mm(a_t[:, s:WQ]), in0=a_t[:, s:WQ],
                            in1=mask_sb[:, 384:896 - s],
                            op=mybir.AluOpType.mult)
                    else:
                        nc.scalar.activation(mm(a_t[:]), s_ps[:],
                                             mybir.ActivationFunctionType.Exp,
                                             bias=shift_sb[:, 0:1], scale=1.0)
                    nc.tensor.matmul(o_ps[:, sq:WQ] if sq else o_ps[:],
                                     mm(vnat[h][b][kb // 4][:, kb % 4, :]),
                                     mm(a_t[:, sq:WQ] if sq else a_t[:]),
                                     start=(kb == 0), stop=(kb == nkb - 1))
                    done_units += 1
                    # sprinkle the previous window's proj work into the
                    # S->exp->PV bubbles on PE
                    if n_c:
                        want = done_units * n_c // n_units
                        while emitted_c < want:
                            next(c_iter)()
                            emitted_c += 1
                den_sb = den_pool.tile([HD, WQ], F32, tag="den",
                                       name="den_sb")
                nc.vector.reciprocal(den_sb[:], o_ps[HD:2 * HD, :])
                nc.vector.tensor_tensor(
                    out=mm(xaw[b * NWB + g][hs, :]), in0=o_ps[0:HD, :],
                    in1=den_sb[:], op=mybir.AluOpType.mult)

        # ---- Stage C for one token window: proj partial for 4 n-blocks ----
        def stage_c_ops(w):
            ops = []
            state = {}

            def mk_mm(half, j, nb, cc):
                def op():
                    if j == 0 and cc == 0:
                        state[half] = y_pool.tile([128, 2, D], F32, tag="y",
                                                  name="y_sb")
                    y_sb = state[half]
                    yp = ps.tile([128, 512], F32, tag="s", name=f"yp{cc}",
                                 bufs=psum_bufs["s"])
                    nc.tensor.matmul(
                        yp[:],
                        mm(xaw[nb // 4][:, (nb % 4) * 128:
                                        (nb % 4 + 1) * 128]),
                        mm(wp_sb[:, cc * 512:(cc + 1) * 512]),
                        start=True, stop=True)
                    if cc == 0:
                        nc.scalar.copy(y_sb[:, j, 0:512], yp[:])
                    else:
                        nc.vector.tensor_copy(y_sb[:, j, 512:1024], yp[:])
                    if j == 1 and cc == 1:
                        nb0 = 4 * w + 2 * half
                        nc.sync.dma_start(
                            out=y[nb0 * 128:(nb0 + 2) * 128, :]
                                .rearrange("(n p) d -> p n d", p=128),
                            in_=y_sb[:])
                return op

            for half in range(2):
                nb0 = 4 * w + 2 * half
                for j, nb in enumerate((nb0, nb0 + 1)):
                    for cc in range(2):
                        ops.append(mk_mm(half, j, nb, cc))
            return ops

        # ---- software-pipelined emission: A(w), then B(w) with the
        # previous window's proj matmuls interleaved into its bubbles ----
        for w in range(NW):
            stage_a(w)
            b, g = divmod(w, NWB)
            stage_b(b, g)
            for op in stage_c_ops(w):
                op()




def _make_mask():
    p = np.arange(128)[:, None]
    j = np.arange(896)[None, :]
    return (j >= p + 384).astype(np.float32)


def _host_scales(W_qkv, u_qkv, sigma_qkv, W_proj, u_proj, sigma_proj):
    """Power-iteration spectral norm in fp32, exactly as the reference:
    v = normalize(W u); sigma = ||W^T v||."""
    def sig(W, u):
        v = (W @ u).astype(np.float32)
        v = v / np.float32(np.linalg.norm(v))
        u2 = (W.T @ v).astype(np.float32)
        return np.float32(np.linalg.norm(u2))
    c_qkv = np.float32(sigma_qkv[0]) / sig(W_qkv, u_qkv)
    c_proj = np.float32(sigma_proj[0]) / sig(W_proj, u_proj)
    return np.float32(c_qkv), np.float32(c_proj)


def make_in_maps(batch, W_qkv, u_qkv, sigma_qkv, W_proj, u_proj, sigma_proj):
    batch = np.asarray(batch, np.float32)
    W_qkv = np.asarray(W_qkv, np.float32)
    u_qkv = np.asarray(u_qkv, np.float32)
    sigma_qkv = np.asarray(sigma_qkv, np.float32)
    W_proj = np.asarray(W_proj, np.float32)
    u_proj = np.asarray(u_proj, np.float32)
    sigma_proj = np.asarray(sigma_proj, np.float32)
    x = np.ascontiguousarray(batch.reshape(NTOK, D))
    # pre-round x to the f32r (tf32-like) grid: the device rounds it at the
    # x^T staging copy anyway, so accuracy is unchanged (~2e-4) and the
    # f32r-typed DMA satisfies the verifier, letting the 256 x-transposes
    # run at 1.5 instead of 2.0 cycles/row
    u = x.view(np.uint32)
    u += ((u >> 13) & 1) + np.uint32((1 << 12) - 1)
    u &= np.uint32(~((1 << 13) - 1) & 0xFFFFFFFF)
    c_qkv, c_proj = _host_scales(W_qkv, u_qkv, sigma_qkv,
                                 W_proj, u_proj, sigma_proj)
    scale = np.float32(HD ** -0.5)
    mask = _make_mask()
    in_maps = []
    for c in range(N_CORES):
        cs = slice(128 * c, 128 * (c + 1))
        in_maps.append({
            "x": x,
            "wq": np.ascontiguousarray(W_qkv[:, cs]),
            "wk": np.ascontiguousarray(W_qkv[:, 1024 + 128 * c:
                                              1024 + 128 * (c + 1)]),
            "wv": np.ascontiguousarray(W_qkv[:, 2048 + 128 * c:
                                              2048 + 128 * (c + 1)]),
            "wp": np.ascontiguousarray(W_proj[cs, :]),
            "cqk": np.full((128, 1), c_qkv * c_qkv * scale, np.float32),
            "cv": np.full((128, 1), c_qkv, np.float32),
            "cp": np.full((128, 1), c_proj, np.float32),
            "mask": mask,
        })
    return in_maps


_NC_CACHE = None


def build_nc():
    global _NC_CACHE
    if _NC_CACHE is None:
        nc = bass.Bass("TRN2", target_bir_lowering=False, debug=False,
                       num_devices=N_CORES)
        with _TileContextSplit(nc) as tc:
            _build_body(nc, tc)
        _NC_CACHE = nc
    return _NC_CACHE


def kernel(batch, W_qkv, u_qkv, sigma_qkv, W_proj, u_proj, sigma_proj):
    in_maps = make_in_maps(batch, W_qkv, u_qkv, sigma_qkv,
                           W_proj, u_proj, sigma_proj)
    nc = build_nc()
    res = run_bass_kernel_spmd(nc, in_maps, list(range(N_CORES)))
    y = np.zeros((NTOK, D), np.float64)
    for c in range(N_CORES):
        y += res.results[c]["y"].astype(np.float64)
    return y.astype(np.float32).reshape(BATCH, NSEQ, D)

